# revision 23
# baseline (speedup 1.0000x reference)
"""Causal single-head attention on 8 Trainium2 NeuronCores.

Problem: x[4096,1024] -> Q,K,V = x@W.T+b (d_k=64), out = softmax(causal(QK^T/8)) @ V.

Strategy (sequence-parallel, uniform SPMD):
  - Query blocks of 128 rows; 32 blocks total. Core c owns global blocks
    {c, c+8, c+16, c+24} (strided) -> every core runs the IDENTICAL program.
  - Each core computes K^T/V~ for its own 512 rows, AllGathers them (split in
    two halves so the second gather overlaps band-0/1 compute), then attends
    its 4 q-blocks against the gathered keys.
  - Band schedule: band s in 0..3 attends q-slots s..3 (N = 512-128s cols)
    against shard-slot s of every rank (8 steps/band). Causality is exact:
    global kb = c'+8s vs qb = c+8j; s<j always valid, s==j masked by a
    per-core host-built mask (ones/triu/zeros by c' vs c), s>j never needed.
  - Softmax denominator comes free: V~ has a ones-column appended, so the
    AV matmul accumulates [out^T; rowsum(E)] in one pass. exp on ScalarE with
    the 1/8 scale folded in; no max-subtraction (scores are O(1) here).
  - float32r (full-rate fp32) matmuls end-to-end; all transposes of inputs
    (x^T, W^T, identity) are host-side numpy; only V^T->V~ (4) and the
    output (4) transpose on-device via PE.
"""

import os
import numpy as np
from contextlib import ExitStack

S, DM, DK = 4096, 1024, 64
NCORES = 8
QB = 128                      # rows per block
SLOTS = 4                     # q-blocks per core
SH = QB * SLOTS               # 512 shard rows per core
# per-half shard (slots 0-1 or 2-3): K^T [64, 256] + V~ [128, 2*65]
KT_H = DK * 2 * QB            # 16384
VT_H = QB * 2 * (DK + 1)      # 16640
SHARD_H = KT_H + VT_H         # 33024

USE_F32R = os.environ.get("KERNEL_F32", "0") != "1"
AMP = int(os.environ.get("KERNEL_AMP", "1"))  # repeat whole pipeline in-NEFF

LAST_EXEC_NS = None


def _build_nc():
    import concourse.bass as bass
    import concourse.bacc as bacc
    import concourse.mybir as mybir
    import concourse.tile as tile

    f32 = mybir.dt.float32
    fr = mybir.dt.float32r if USE_F32R else mybir.dt.float32
    AF = mybir.ActivationFunctionType

    nc = bacc.Bacc(None, num_devices=NCORES)

    xT_d = nc.dram_tensor("xT", [DM, SH], fr, kind="ExternalInput")
    wqkT_d = nc.dram_tensor("wqkT", [DM, 2 * DK], fr, kind="ExternalInput")
    wvT_d = nc.dram_tensor("wvT", [DM, DK], fr, kind="ExternalInput")
    bqk_d = nc.dram_tensor("bqk", [2 * DK, 1], f32, kind="ExternalInput")
    bv_d = nc.dram_tensor("bv", [DK, 1], f32, kind="ExternalInput")
    mask_d = nc.dram_tensor("mask", [NCORES * QB, SH], fr, kind="ExternalInput")
    tri_d = nc.dram_tensor("tri", [QB, QB], fr, kind="ExternalInput")
    ident_d = nc.dram_tensor("ident", [128, 128], fr, kind="ExternalInput")
    out_d = nc.dram_tensor("out", [SH, DK], f32, kind="ExternalOutput")

    with tile.TileContext(nc) as tc, ExitStack() as ctx:
        singles = ctx.enter_context(tc.tile_pool(name="singles", bufs=1))
        psum = ctx.enter_context(tc.tile_pool(name="psum", bufs=1, space="PSUM"))
        psum2 = ctx.enter_context(tc.tile_pool(name="psum2", bufs=2, space="PSUM"))
        kvpool = ctx.enter_context(tc.tile_pool(name="kvpool", bufs=3))
        epool = ctx.enter_context(tc.tile_pool(name="epool", bufs=3))
        dram = ctx.enter_context(tc.tile_pool(name="dram", bufs=1, space="DRAM"))

        # ---------------- input loads (small constants first) ----------------
        wqk_sb = singles.tile([128, DM // 128, 2 * DK], fr)
        nc.sync.dma_start(out=wqk_sb, in_=wqkT_d[:, :].rearrange("(d p) c -> p d c", p=128))
        wv_sb = singles.tile([128, DM // 128, DK], fr)
        nc.sync.dma_start(out=wv_sb, in_=wvT_d[:, :].rearrange("(d p) c -> p d c", p=128))
        bqk_sb = singles.tile([128, 1], f32)
        nc.sync.dma_start(out=bqk_sb, in_=bqk_d[:, :])
        bv_sb = singles.tile([64, 1], f32)
        nc.sync.dma_start(out=bv_sb, in_=bv_d[:, :])
        mask_sb = singles.tile([128, NCORES, SH], fr)
        tri_sb = singles.tile([128, QB], fr)
        ident_fr = singles.tile([128, 128], fr)
        nc.sync.dma_start(out=ident_fr, in_=ident_d[:, :])

        xT_sb = singles.tile([128, DM // 128, SH], fr)
        qkT_sb = singles.tile([128, SH], fr)
        vt_sb = singles.tile([128, SLOTS, DK + 1], fr)
        # ones column of V~ (f32r memset is invalid ISA; ACT writes 0*x+1)
        nc.scalar.activation(vt_sb[:, :, DK:DK + 1], ident_fr[:, 0:SLOTS].bitcast(f32),
                             AF.Identity, bias=1.0, scale=0.0)
        def load_xt_half(h):
            cs = slice(256 * h, 256 * (h + 1))
            for q in range(2):
                nc.sync.dma_start(
                    out=xT_sb[:, 4 * q:4 * (q + 1), cs],
                    in_=xT_d[512 * q:512 * (q + 1), cs].rearrange(
                        "(d p) s -> p d s", p=128))

        rep_counter = [0]

        def band_kt_ap(ag_out, s):
            t = ag_out[s // 2]
            return bass.AP(tensor=t.tensor, offset=t.offset + QB * (s % 2),
                           ap=[[2 * QB, DK], [SHARD_H, NCORES], [1, QB]])

        def band_vt_ap(ag_out, s):
            t = ag_out[s // 2]
            return bass.AP(tensor=t.tensor,
                           offset=t.offset + KT_H + (DK + 1) * (s % 2),
                           ap=[[2 * (DK + 1), QB], [SHARD_H, NCORES], [1, DK + 1]])

        def one_pass():
            # ------------- per-half: project, build V~, AllGather -------------
            r = rep_counter[0]
            rep_counter[0] += 1
            ag_in = [dram.tile([SHARD_H], fr, name=f"ag_in{r}_{h}",
                               tag=f"agi{r}_{h}") for h in range(2)]
            ag_out = [dram.tile([NCORES * SHARD_H], fr, addr_space="Shared",
                                name=f"ag_out{r}_{h}", tag=f"ago{r}_{h}")
                      for h in range(2)]
            load_xt_half(0)
            for h in range(2):
                cols = slice(256 * h, 256 * (h + 1))
                qk_ps = psum2.tile([128, 256], f32, tag="sc", bufs=3, name="qk_ps")
                v_ps = psum2.tile([64, 256], f32, tag="sc", bufs=3, name="v_ps")
                for d in range(DM // 128):
                    nc.tensor.matmul(qk_ps, lhsT=wqk_sb[:, d, :],
                                     rhs=xT_sb[:, d, cols],
                                     start=(d == 0), stop=(d == DM // 128 - 1))
                    nc.tensor.matmul(v_ps, lhsT=wv_sb[:, d, :],
                                     rhs=xT_sb[:, d, cols],
                                     start=(d == 0), stop=(d == DM // 128 - 1))
                nc.scalar.activation(qkT_sb[:, cols], qk_ps, AF.Identity,
                                     bias=bqk_sb[:, 0:1], scale=1.0)
                nc.sync.dma_start(
                    out=ag_in[h][0:KT_H].rearrange("(p s) -> p s", p=DK),
                    in_=qkT_sb[64:128, cols])
                vT_h = epool.tile([64, 256], fr, tag="vth", name="vT_h")
                nc.scalar.activation(vT_h, v_ps, AF.Identity,
                                     bias=bv_sb[:, 0:1], scale=1.0)
                for sl in range(2):
                    t_ps = psum2.tile([128, 64], fr, tag="tps", bufs=1, name="t_ps")
                    nc.tensor.transpose(t_ps, vT_h[:, 128 * sl:128 * (sl + 1)],
                                        ident_fr[0:64, 0:64])
                    nc.scalar.copy(vt_sb[:, 2 * h + sl, 0:DK], t_ps)
                nc.sync.dma_start(
                    out=ag_in[h][KT_H:SHARD_H].rearrange("(p a) -> p a", p=128),
                    in_=vt_sb[:, 2 * h:2 * (h + 1), :].rearrange("p a b -> p (a b)"))
                nc.gpsimd.collective_compute(
                    "AllGather", mybir.AluOpType.bypass,
                    replica_groups=[list(range(NCORES))],
                    ins=[ag_in[h][:]], outs=[ag_out[h][:]],
                )
                if h == 0:
                    load_xt_half(1)
                    if r == 0:
                        # 2MB of masks: behind both x^T halves; first needed
                        # only by the prepass multiplies
                        nc.sync.dma_start(
                            out=mask_sb,
                            in_=mask_d[:, :].rearrange("(c p) q -> p c q", p=128))
                        nc.sync.dma_start(out=tri_sb, in_=tri_d[:, :])
                    av_ps = psum.tile([DK + 1, SH], f32, name="av_ps")
                    for s in range(2):
                        c0, N = 128 * s, 256 - 128 * s
                        lsc = psum2.tile([128, 2, 512], f32, tag="sc", bufs=3, name="lscA")
                        le = epool.tile([128, 2, 512], fr, tag="e", name="leA")
                        lkt = kvpool.tile([DK, QB], fr, tag="lkt", name="lktA")
                        nc.sync.dma_start(
                            out=lkt,
                            in_=ag_in[0][0:KT_H].rearrange(
                                "(p s) -> p s", p=DK)[:, QB * s:QB * (s + 1)])
                        nc.tensor.matmul(lsc[:, 0, 0:N], lhsT=lkt,
                                         rhs=qkT_sb[0:64, c0:256],
                                         start=True, stop=True)
                        nc.scalar.activation(le[:, 0, 0:N], lsc[:, 0, 0:N],
                                             AF.Exp, scale=0.125)
                        nc.vector.tensor_mul(le[:, 0, 0:QB], le[:, 0, 0:QB],
                                             tri_sb)
                        nc.tensor.matmul(av_ps[:, c0:256], lhsT=vt_sb[:, s, :],
                                         rhs=le[:, 0, 0:N], start=(s == 0),
                                         stop=False, skip_group_check=True)


            # ---- local prepass part B: own blocks vs Q cols 256:512 ----
            # (part A ran inside the h-loop right after half 0; see below)
            for s in range(SLOTS):
                c0 = max(256, 128 * s)
                N = SH - c0
                lsc = psum2.tile([128, 2, 512], f32, tag="sc", bufs=3, name="lscB")
                le = epool.tile([128, 2, 512], fr, tag="e", name="leB")
                lkt = kvpool.tile([DK, QB], fr, tag="lkt", name="lktB")
                nc.sync.dma_start(
                    out=lkt,
                    in_=ag_in[s // 2][0:KT_H].rearrange(
                        "(p s) -> p s", p=DK)[:, QB * (s % 2):QB * (s % 2 + 1)])
                nc.tensor.matmul(lsc[:, 0, 0:N], lhsT=lkt,
                                 rhs=qkT_sb[0:64, c0:SH], start=True, stop=True)
                nc.scalar.activation(le[:, 0, 0:N], lsc[:, 0, 0:N], AF.Exp,
                                     scale=0.125)
                if s >= 2:   # diagonal strip lies in these columns
                    nc.vector.tensor_mul(le[:, 0, 0:QB], le[:, 0, 0:QB], tri_sb)
                nc.tensor.matmul(av_ps[:, c0:SH], lhsT=vt_sb[:, s, :],
                                 rhs=le[:, 0, 0:N], start=False, stop=False,
                                 skip_group_check=True)

            # ---------------- attention bands ----------------
            first_av = False
            for s in range(SLOTS):
                N = SH - 128 * s
                q_ap = qkT_sb[0:64, 128 * s:SH]
                ktb = kvpool.tile([DK, NCORES, QB], fr, tag="ktb", name="ktb")
                vtb = kvpool.tile([QB, NCORES, DK + 1], fr, tag="vtb", name="vtb")
                nc.sync.dma_start(out=ktb, in_=band_kt_ap(ag_out, s))
                nc.sync.dma_start(out=vtb, in_=band_vt_ap(ag_out, s))
                W = 2 if s < 2 else 4       # steps per exp; N<=256 fits 4/tile
                for g in range(NCORES // W):
                    sc_ps = psum2.tile([128, W, 1024 // W], f32, tag="sc",
                                       bufs=3, name="sc_ps")
                    e_sb = epool.tile([128, W, 1024 // W], fr, tag="e", name="e_sb")
                    for hh in range(W):
                        cp = W * g + hh
                        nc.tensor.matmul(sc_ps[:, hh, 0:N], lhsT=ktb[:, cp, :],
                                         rhs=q_ap, start=True, stop=True)
                    nc.scalar.activation(e_sb[:, :, 0:N], sc_ps[:, :, 0:N], AF.Exp,
                                         scale=0.125)
                    for hh in range(W):
                        cp = W * g + hh
                        nc.vector.tensor_mul(e_sb[:, hh, 0:N], e_sb[:, hh, 0:N],
                                             mask_sb[:, cp, 0:N])
                        last_av = (s == SLOTS - 1 and g == NCORES // W - 1
                                   and hh == W - 1)
                        nc.tensor.matmul(av_ps[:, 128 * s:SH], lhsT=vtb[:, cp, :],
                                         rhs=e_sb[:, hh, 0:N],
                                         start=first_av, stop=last_av,
                                         skip_group_check=True)
                        first_av = False

            # ------------- epilogue: transpose, normalize, store -------------
            av_sb = singles.tile([DK + 1, SH], f32, name="av_sb")
            nc.scalar.copy(av_sb, av_ps)
            out_sb = singles.tile([128, SLOTS, DK], f32, name="out_sb")
            for sl in range(SLOTS):
                t2 = psum2.tile([128, DK + 1], f32, tag="tps", bufs=1, name="t2")
                nc.tensor.transpose(t2, av_sb[0:DK + 1, 128 * sl:128 * (sl + 1)],
                                    ident_fr[0:DK + 1, 0:DK + 1].bitcast(f32))
                rec = epool.tile([128, 1], f32, tag="rec", name="rec")
                nc.vector.reciprocal(rec, t2[:, DK:DK + 1])
                nc.vector.tensor_scalar_mul(out_sb[:, sl, :], t2[:, 0:DK], rec)
                nc.sync.dma_start(out=out_d[128 * sl:128 * (sl + 1), :],
                                  in_=out_sb[:, sl, :])

        for _rep in range(AMP):
            one_pass()

    nc.finalize()
    return nc


def _in_maps(x, Wq, bq, Wk, bk, Wv, bv):
    wqkT = np.ascontiguousarray(np.concatenate([Wq.T, Wk.T], axis=1), dtype=np.float32)
    wvT = np.ascontiguousarray(Wv.T, dtype=np.float32)
    bqk = np.concatenate([bq, bk]).reshape(2 * DK, 1).astype(np.float32)
    bvv = bv.reshape(DK, 1).astype(np.float32)
    tri = np.triu(np.ones((QB, QB), dtype=np.float32))  # E^T[k,q] valid iff k<=q
    maps = []
    for c in range(NCORES):
        rows = np.concatenate([np.arange(QB * (c + 8 * sl), QB * (c + 8 * sl) + QB)
                               for sl in range(SLOTS)])
        xT = np.ascontiguousarray(x[rows].T, dtype=np.float32)  # [1024, 512]
        # [c', k, q-col] over the full 512-col band window. strip = first 128
        # cols (q-slot s); own position contributes via the local prepass.
        m = np.zeros((NCORES, QB, SH), dtype=np.float32)
        m[:c] = 1.0                   # earlier ranks: fully valid
        m[c + 1:, :, QB:] = 1.0       # later ranks: valid beyond the strip
        maps.append({
            "xT": xT, "wqkT": wqkT, "wvT": wvT, "bqk": bqk, "bv": bvv,
            "mask": np.ascontiguousarray(m.reshape(NCORES * QB, SH)),
            "tri": tri, "ident": np.eye(128, dtype=np.float32),
        })
    return maps


def kernel(**inputs):
    global LAST_EXEC_NS
    x = np.asarray(inputs["x"], dtype=np.float32)
    args = [np.asarray(inputs[k], dtype=np.float32)
            for k in ("Wq", "bq", "Wk", "bk", "Wv", "bv")]
    in_maps = _in_maps(x, args[0], args[1], args[2], args[3], args[4], args[5])

    nc = _build_nc()
    from concourse.bass_utils import run_bass_kernel_spmd
    res = run_bass_kernel_spmd(nc, in_maps, core_ids=list(range(NCORES)))
    LAST_EXEC_NS = res.exec_time_ns

    out = np.zeros((S, DK), dtype=np.float32)
    for c in range(NCORES):
        r = res.results[c]["out"]
        for sl in range(SLOTS):
            b = c + 8 * sl
            out[QB * b:QB * (b + 1)] = r[QB * sl:QB * (sl + 1)]
    return out
